# revision 1
# baseline (speedup 1.0000x reference)
"""Multi-Head Latent Attention kernel for 8 Trainium2 NeuronCores.

Sharding: 8 cores = 2 (batch) x 4 (head groups of 4 heads).

MLA weight absorption (per core, head group g):
  scores_h = (x @ W'_h + b'_h) @ kv^T   with W'_h = Wq_h Wk_h^T / 8  [1024,16]
  (k-bias and kv-bias drop out of softmax; all v-path biases fold into a
   host-side output bias: bo + (bc Wv + bv) Wo)
  z_h  = softmax_num @ [1 | kv]         (denominator via ones column)
  out  = (z_h / den_h) @ W2s            with W2 = Wv_h Wo_h stacked [16,1024]

Engine plan:
  PE:   proj (bf16, one [128,96] stationary pass over xT), scores (fp8e4
        DoubleRow, zero-padded K=32 lhsT so all partition bases stay
        32-aligned), z (bf16, M=32 so junk psum rows hold the denominator),
        recip broadcast (bf16 sel matmul), out-proj (K=128 over strip-layout
        ZTs with zero rows in W2s).
  ACT:  exp of scores (bulk), psum->sbuf out drains (some).
  DVE:  casts to fp8/bf16, causal tri mask, recip + norm mul, drains,
        Schraudolph exp offload (uint16 bitcast) for selected pairs.
  Pool: sbuf-only memsets and bf16 recip cast (GPSIMD cannot touch PSUM).
"""
import sys
import math

sys.path.insert(0, "/opt/trn_rl_repo")

import numpy as np
import ml_dtypes

import concourse.bass as bass
import concourse.tile as tile
from concourse import bacc, mybir
from concourse.bass_utils import run_bass_kernel_spmd

BF16 = ml_dtypes.bfloat16
E4M3 = ml_dtypes.float8_e4m3fn

# Problem shape (hardcoded per contract)
B, T, D = 2, 2048, 1024
H = 16
HD = 64
KV = 16
HPC = 4            # heads per core
SCALE = 1.0 / math.sqrt(HD)
NB = T // 128      # key blocks = 16

F32 = mybir.dt.float32
BF = mybir.dt.bfloat16
FP8 = mybir.dt.float8e4
U16 = mybir.dt.uint16
EXP = mybir.ActivationFunctionType.Exp
DR = mybir.MatmulPerfMode.DoubleRow

# Schraudolph exp constants (bf16 bit trick): bits = s*SCH_A + SCH_B
SCH_A = float(2.0 ** 7 / math.log(2.0))
SCH_B = float(127.0 * 2 ** 7 - 4.8)

# (qc, h, p) pairs whose exp runs on DVE via the bit trick instead of ACT.
# Exp pairs on DVE: the short diagonal pair of every (window, head) --
# its tri mask then runs on the same queue (no cross-engine wait) -- plus
# every 3rd non-diagonal pair for load balance.
OFFLOAD = {(qc, h, 2 * qc + 1) for qc in range(4) for h in range(4)} | \
          {(qc, h, p) for qc in range(4) for h in range(4)
           for p in range(2 * qc) if (p + h) % 3 == 2}

_CACHE = {}


def _build_program():
    nc = bacc.Bacc("TRN2", target_bir_lowering=False, debug=False)

    xT = nc.dram_tensor("xT", [D, T], BF, kind="ExternalInput")
    wcq = nc.dram_tensor("wcq", [128, 8, 96], BF, kind="ExternalInput")
    bprime = nc.dram_tensor("bprime", [64, 1], F32, kind="ExternalInput")
    w2s = nc.dram_tensor("w2s", [128, D], BF, kind="ExternalInput")
    sel = nc.dram_tensor("sel", [128, 128], F32, kind="ExternalInput")
    tri = nc.dram_tensor("tri", [128, 128], BF, kind="ExternalInput")
    id16 = nc.dram_tensor("id16", [16, 16], BF, kind="ExternalInput")
    outp = nc.dram_tensor("outp", [T, D], BF, kind="ExternalOutput")

    with tile.TileContext(nc) as tc:
        with (
            tc.tile_pool(name="const", bufs=1) as const,
            tc.tile_pool(name="work", bufs=2) as work,
            tc.tile_pool(name="pxp", bufs=12) as pxp,
            tc.tile_pool(name="ps", bufs=2, space="PSUM") as ps,
        ):
            # ---- constants (xT slab 0 first so proj can start asap) ----
            wcq_sb = const.tile([128, 8, 96], BF)
            nc.sync.dma_start(out=wcq_sb, in_=wcq.ap())
            xT_sb = const.tile([128, 8, T], BF)
            xT_r = xT.ap().rearrange("(k p) t -> p k t", p=128)
            dmae = [nc.sync, nc.gpsimd]
            for kt in range(8):
                dmae[kt % 2].dma_start(out=xT_sb[:, kt, 0:512],
                                       in_=xT_r[:, kt, 0:512])
            bprime_sb = const.tile([64, 1], F32)
            nc.gpsimd.dma_start(out=bprime_sb, in_=bprime.ap())
            id16_sb = const.tile([16, 16], BF)
            nc.gpsimd.dma_start(out=id16_sb, in_=id16.ap())
            # remaining slabs: split each across both queues for 2x bandwidth
            for s in range(1, 4):
                sl = slice(512 * s, 512 * s + 512)
                nc.sync.dma_start(out=xT_sb[:, 0:4, sl],
                                  in_=xT_r[:, 0:4, sl])
                nc.gpsimd.dma_start(out=xT_sb[:, 4:8, sl],
                                    in_=xT_r[:, 4:8, sl])
            w2s_sb = const.tile([128, D], BF)
            nc.sync.dma_start(out=w2s_sb, in_=w2s.ap())
            self_f32 = const.tile([128, 128], F32)
            nc.sync.dma_start(out=self_f32, in_=sel.ap())
            tri_sb = const.tile([128, 128], BF)
            nc.gpsimd.dma_start(out=tri_sb, in_=tri.ap())

            # persistent activation tensors
            # scores lhsT: per-head K=64 zero-padded kv (kv at rows
            # 16h..16h+16).  K=64 tiles run at full PE rate; K=32 tiles are
            # half rate.  All heads share the q8[0:64] rhs; zero lhsT rows
            # select the head.
            kv64 = [const.tile([64, T], BF, name=f"kv64_{h}")
                    for h in range(4)]
            q8 = const.tile([64, T], BF)       # q' for 4 heads (16 rows each)
            kv_aug = const.tile([128, NB, 32], BF)  # col0 ones, 1:17 kv, rest 1
            ZTs = const.tile([128, T], BF)
            outstage = const.tile([128, 16, D], BF)

            for h in range(4):
                nc.vector.memset(kv64[h], 0.0)
            nc.gpsimd.memset(kv_aug[:, :, 0:1], 1.0)
            nc.gpsimd.memset(kv_aug[:, :, 17:32], 1.0)

            # ---- emission units ----
            def proj(s):
                """Project slab s (512 tokens): kv + q' for 4 heads."""
                sl = slice(512 * s, 512 * s + 512)
                pp = ps.tile([96, 512], F32, tag="po2", bufs=2, name=f"pp{s}")
                for kt in range(8):
                    nc.tensor.matmul(pp, lhsT=wcq_sb[:, kt, :],
                                     rhs=xT_sb[:, kt, sl],
                                     start=(kt == 0), stop=(kt == 7))
                nc.scalar.copy(out=kv64[0][0:16, sl], in_=pp[0:16, :])
                nc.vector.tensor_scalar_add(q8[0:32, sl], pp[32:64, :],
                                            bprime_sb[0:32, :])
                nc.vector.tensor_scalar_add(q8[32:64, sl], pp[64:96, :],
                                            bprime_sb[32:64, :])
                # per-head lhsT copies (sbuf->sbuf DMA, no alignment rules)
                nc.sync.dma_start(out=kv64[1][16:32, sl],
                                  in_=kv64[0][0:16, sl])
                nc.sync.dma_start(out=kv64[2][32:48, sl],
                                 in_=kv64[0][0:16, sl])
                nc.sync.dma_start(out=kv64[3][48:64, sl],
                                  in_=kv64[0][0:16, sl])
                # keys-major kv for the z matmuls, via PE transpose
                tp = ps.tile([128, 4, 16], BF, tag="po2", bufs=2, name=f"tp{s}")
                for j in range(4):
                    b = 4 * s + j
                    nc.tensor.transpose(tp[:, j, :],
                                        in_=kv64[0][0:16, 128 * b:128 * b + 128],
                                        identity=id16_sb)
                nc.vector.tensor_copy(out=kv_aug[:, 4 * s:4 * s + 4, 1:17],
                                      in_=tp)

            pxs = {}

            def score_pair(qc, h, p):
                """Scores + exp for key-block pair p of (window qc, head h)."""
                q0 = 512 * qc
                vs = max(0, 256 * p - q0)
                sc = ps.tile([128, 2, 512], F32, tag="sc", bufs=2,
                             name=f"sc{qc}{h}{p}")
                for j in range(2):
                    b = 2 * p + j
                    vb = max(0, 128 * b - q0)
                    nc.tensor.matmul(
                        sc[:, j, vb:512],
                        lhsT=kv64[h][:, 128 * b:128 * b + 128],
                        rhs=q8[:, q0 + vb:q0 + 512],
                        start=True, stop=True,
                        tile_position=(0, 0))
                px = pxp.tile([128, 2, 512], BF, tag="px", name=f"px{qc}{h}{p}")
                pxs[(qc, h, p)] = px
                if (qc, h, p) in OFFLOAD:
                    nc.vector.tensor_scalar(
                        out=px[:, :, vs:512].bitcast(U16),
                        in0=sc[:, :, vs:512],
                        scalar1=SCH_A, scalar2=SCH_B,
                        op0=mybir.AluOpType.mult, op1=mybir.AluOpType.add)
                else:
                    nc.scalar.activation(px[:, :, vs:512], sc[:, :, vs:512],
                                         EXP)
                if p >= 2 * qc:  # diagonal pair: causal stair mask
                    stair = bass.AP(
                        tensor=px.tensor,
                        offset=px[:, 0, vs:vs + 128].offset,
                        ap=[list(px.ap)[0], [512 + 128, 2], [1, 128]])
                    trib = bass.AP(
                        tensor=tri_sb.tensor, offset=tri_sb.offset,
                        ap=[list(tri_sb.ap)[0], [0, 2], [1, 128]])
                    eng = nc.vector if (qc, h, p) in OFFLOAD else nc.gpsimd
                    eng.tensor_mul(stair, stair, trib)

            def z_pair(qc, h, p, blocks):
                q0 = 512 * qc
                px = pxs.pop((qc, h, p))
                zp = zps[qc]
                for j in range(2):
                    b = 2 * p + j
                    vb = max(0, 128 * b - q0)
                    nc.tensor.matmul(
                        zp[32 * h:32 * h + 32, vb:512],
                        lhsT=kv_aug[:, b, :],
                        rhs=px[:, j, vb:512],
                        start=(b == 0), stop=(b == blocks - 1),
                        tile_position=(0, 32 * h))

            def normchain(qc, c0=0, c1=512):
                q0 = 512 * qc
                zp = zps[qc]
                w = c1 - c0
                recf = work.tile([128, 512], F32, tag="recf",
                                 name=f"rf{qc}{c0}")
                nc.vector.reciprocal_approx_fast(out=recf[:, 0:w],
                                                 in_=zp[:, c0:c1])
                rbc = ps.tile([128, 512], F32, tag="po2", bufs=2,
                              name=f"rbc{qc}{c0}")
                nc.tensor.matmul(rbc[:, 0:w], lhsT=self_f32,
                                 rhs=recf[:, 0:w], start=True, stop=True)
                rbcs = work.tile([128, 512], F32, tag="rbcs",
                                 name=f"rs{qc}{c0}")
                nc.vector.tensor_copy(out=rbcs[:, 0:w], in_=rbc[:, 0:w])
                nc.vector.tensor_mul(ZTs[:, q0 + c0:q0 + c1], zp[:, c0:c1],
                                     rbcs[:, 0:w])

            def outproj_unit(qc, m, n):
                qt = 4 * qc + m
                po = ps.tile([128, 512], F32, tag="po2", bufs=2,
                             name=f"po{qc}{m}{n}")
                nc.tensor.matmul(
                    po, lhsT=ZTs[:, 128 * qt:128 * qt + 128],
                    rhs=w2s_sb[:, 512 * n:512 * n + 512],
                    start=True, stop=True)
                if (m + n) % 2 == 0:
                    nc.scalar.copy(out=outstage[:, qt, 512 * n:512 * n + 512],
                                   in_=po)
                else:
                    nc.vector.tensor_copy(
                        out=outstage[:, qt, 512 * n:512 * n + 512], in_=po)

            def outdma(qc):
                out_r = outp.ap().rearrange("(m p) n -> p m n", p=128)
                nc.sync.dma_start(
                    out=out_r[:, 4 * qc:4 * qc + 4, :],
                    in_=outstage[:, 4 * qc:4 * qc + 4, :])

            # ---- schedule ----
            zps = {}
            for qc in range(4):
                zps[qc] = ps.tile([128, 512], F32, tag="zb", bufs=2,
                                  name=f"zps{qc}")

            proj(0)

            LAG = 10
            for qc in range(4):
                blocks = 4 * qc + 4
                pairs = 2 * qc + 2
                sunits = [(h, p) for h in range(4) for p in range(pairs)]
                units = []
                for i, (h, p) in enumerate(sunits):
                    units.append(lambda h=h, p=p: score_pair(qc, h, p))
                    if i >= LAG:
                        hz, pz = sunits[i - LAG]
                        units.append(
                            lambda h=hz, p=pz: z_pair(qc, h, p, blocks))
                for h, p in sunits[-LAG:]:
                    units.append(lambda h=h, p=p: z_pair(qc, h, p, blocks))
                fillers = []
                if qc == 0:
                    fillers += [lambda: proj(1), lambda: proj(2)]
                if qc == 1:
                    fillers.append(lambda: proj(3))
                if qc >= 1:
                    # first: previous window's normalization (zb ring frees),
                    # then its out-projection units
                    fillers.append(lambda q=qc - 1: normchain(q))
                    fillers += [
                        lambda m=m, n=n, q=qc - 1: outproj_unit(q, m, n)
                        for m in range(4) for n in range(2)
                    ]
                stride = max(1, len(units) // max(1, len(fillers)))
                if qc == 0:
                    stride = 2   # front-load proj fillers behind their DMAs
                fi = 0
                for ui, u in enumerate(units):
                    u()
                    if fi < len(fillers) and ui % stride == stride - 1:
                        fillers[fi]()
                        fi += 1
                while fi < len(fillers):
                    fillers[fi]()
                    fi += 1
                if qc >= 1:
                    outdma(qc - 1)

            normchain(3, 0, 256)
            for m in (0, 1):
                for n in range(2):
                    outproj_unit(3, m, n)
            normchain(3, 256, 512)
            out_r3 = outp.ap().rearrange("(m p) n -> p m n", p=128)
            nc.sync.dma_start(out=out_r3[:, 12:14, :],
                              in_=outstage[:, 12:14, :])
            for m in (2, 3):
                for n in range(2):
                    outproj_unit(3, m, n)
            nc.sync.dma_start(out=out_r3[:, 14:16, :],
                              in_=outstage[:, 14:16, :])

    nc.compile()
    return nc


def _prep_inputs(inputs):
    x = np.asarray(inputs["x"], np.float32)
    Wc = np.asarray(inputs["Wc"], np.float32)
    bc = np.asarray(inputs["bc"], np.float32)
    Wk = np.asarray(inputs["Wk"], np.float32)
    Wv = np.asarray(inputs["Wv"], np.float32)
    bv = np.asarray(inputs["bv"], np.float32)
    Wq = np.asarray(inputs["Wq"], np.float32)
    bq = np.asarray(inputs["bq"], np.float32)
    Wo = np.asarray(inputs["Wo"], np.float32)

    tri_np = np.triu(np.ones((128, 128), np.float32)).astype(BF16)
    id16_np = np.eye(16, dtype=BF16)
    sel_np = np.zeros((128, 128), np.float32)
    for h in range(HPC):
        sel_np[32 * h, 32 * h:32 * h + 17] = 1.0

    xT_np = [np.ascontiguousarray(x[b].T).astype(BF16) for b in range(B)]

    in_maps = []
    for core in range(8):
        b, g = core // 4, core % 4
        # wcq: [1024, 96] -> kt-major [128, 8, 96]
        wcq_np = np.zeros((D, 96), np.float32)
        wcq_np[:, 0:16] = Wc
        bprime_np = np.zeros((64,), np.float32)
        w2s_np = np.zeros((128, D), np.float32)
        for h in range(HPC):
            gh = 4 * g + h
            hs = slice(HD * gh, HD * gh + HD)
            wcq_np[:, 32 + 16 * h:48 + 16 * h] = (Wq[:, hs] * SCALE) @ Wk[:, hs].T
            bprime_np[16 * h:16 * h + 16] = (bq[hs] * SCALE) @ Wk[:, hs].T
            w2s_np[32 * h + 1:32 * h + 17, :] = Wv[:, hs] @ Wo[hs, :]
        wcq_np = np.ascontiguousarray(
            wcq_np.reshape(8, 128, 96).transpose(1, 0, 2)).astype(BF16)
        in_maps.append({
            "xT": xT_np[b],
            "wcq": wcq_np,
            "bprime": bprime_np.reshape(64, 1),
            "w2s": w2s_np.astype(BF16),
            "sel": sel_np,
            "tri": tri_np,
            "id16": id16_np,
        })
    return in_maps


def run(inputs, trace=False, tmpdir=None):
    if "nc" not in _CACHE:
        _CACHE["nc"] = _build_program()
    nc = _CACHE["nc"]
    in_maps = _prep_inputs(inputs)

    kwargs = {}
    if trace:
        try:
            import antenv.axon_hooks  # noqa: F401
        except ImportError:
            import types
            import antenv  # noqa: F401
            from trn_agent_boot.trn_boot import _ntff_profile_via_ctypes
            hook = _ntff_profile_via_ctypes("/opt/axon/libaxon_pjrt.so")
            mod = types.ModuleType("antenv.axon_hooks")
            mod.get_axon_ntff_profile_hook = lambda: hook
            sys.modules["antenv.axon_hooks"] = mod
        kwargs = dict(trace=True, tmpdir=tmpdir)

    res = run_bass_kernel_spmd(nc, in_maps, list(range(8)), **kwargs)

    bc = np.asarray(inputs["bc"], np.float32)
    Wv = np.asarray(inputs["Wv"], np.float32)
    bv = np.asarray(inputs["bv"], np.float32)
    Wo = np.asarray(inputs["Wo"], np.float32)
    bo = np.asarray(inputs["bo"], np.float32)
    host_bias = bo + (bc @ Wv + bv) @ Wo

    out = np.zeros((B, T, D), np.float32)
    for core in range(8):
        out[core // 4] += res.results[core]["outp"].astype(np.float32)
    out += host_bias
    return out, res


def kernel(**inputs):
    out, _ = run(inputs, trace=False)
    return out



# revision 10
# speedup vs baseline: 1.0425x; 1.0425x over previous
"""Multi-Head Latent Attention kernel for 8 Trainium2 NeuronCores.

Sharding: 8 cores = 2 (batch) x 4 (head groups of 4 heads).

MLA weight absorption (per core, head group g):
  scores_h = (x @ W'_h + b'_h) @ kv^T   with W'_h = Wq_h Wk_h^T / 8  [1024,16]
  z_h  = softmax_num @ [1 | kv]         (denominator via ones column)
  out  = (z_h / den_h) @ W2s            with W2 = Wv_h Wo_h stacked [16,1024]

Engine plan (v2 -- PE array tiling + saturated PSUM drains):
  PE:   proj (bf16, K=128 full array, bias folded in via ones-row matmul),
        scores as 4 CONCURRENT row-tiled matmuls (tile_position=(32h,0),
        K=32 strips, one per head), z as 4 CONCURRENT col-tiled matmuls
        (tile_position=(0,32h), M=32 strips), out-proj full array.
  ACT:  exp of heads 0/1 score units (PSUM->SBUF fused), q128 copies,
        kv_aug drains, most out-proj drains.
  DVE:  exp of heads 2/3 units (Schraudolph bit trick), norm chain
        (reciprocal + stream_shuffle den broadcast + mul), rest of drains.
  Pool: causal stair masks on SBUF px tiles, memsets, input DMA queue.
"""
import sys
import math

sys.path.insert(0, "/opt/trn_rl_repo")

import numpy as np
import ml_dtypes

import concourse.bass as bass
import concourse.tile as tile
from concourse import bacc, mybir
from concourse.bass_utils import run_bass_kernel_spmd

BF16 = ml_dtypes.bfloat16

# Problem shape (hardcoded per contract)
B, T, D = 2, 2048, 1024
H = 16
HD = 64
KV = 16
HPC = 4            # heads per core
SCALE = 1.0 / math.sqrt(HD)
NB = T // 128      # key blocks = 16

F32 = mybir.dt.float32
BF = mybir.dt.bfloat16
U16 = mybir.dt.uint16
EXP = mybir.ActivationFunctionType.Exp

# Schraudolph exp constants (bf16 bit trick): bits = s*SCH_A + SCH_B
SCH_A = float(2.0 ** 7 / math.log(2.0))
SCH_B = float(127.0 * 2 ** 7 - 4.8)

_CACHE = {}


def _build_program():
    nc = bacc.Bacc("TRN2", target_bir_lowering=False, debug=False)

    xT = nc.dram_tensor("xT", [D, T], BF, kind="ExternalInput")
    wcq = nc.dram_tensor("wcq", [128, 8, 128], BF, kind="ExternalInput")
    bpr = nc.dram_tensor("bpr", [1, 128], BF, kind="ExternalInput")
    w2s = nc.dram_tensor("w2s", [128, D], BF, kind="ExternalInput")
    tri = nc.dram_tensor("tri", [128, 128], BF, kind="ExternalInput")
    id16 = nc.dram_tensor("id16", [16, 16], BF, kind="ExternalInput")
    outp = nc.dram_tensor("outp", [T, D], BF, kind="ExternalOutput")

    with tile.TileContext(nc) as tc:
        with (
            tc.tile_pool(name="const", bufs=1) as const,
            tc.tile_pool(name="work", bufs=2) as work,
            tc.tile_pool(name="pxp", bufs=8) as pxp,
            tc.tile_pool(name="ps", bufs=2, space="PSUM") as ps,
        ):
            # ---- constants (wcq first so PE/ACT warmups start asap) ----
            wcq_sb = const.tile([128, 8, 128], BF)
            nc.sync.dma_start(out=wcq_sb, in_=wcq.ap())
            xT_sb = const.tile([128, 8, T], BF)
            xT_r = xT.ap().rearrange("(k p) t -> p k t", p=128)
            # slab 0 in two big transfers (one per queue), rest after
            nc.sync.dma_start(out=xT_sb[:, 0:4, 0:512], in_=xT_r[:, 0:4, 0:512])
            nc.gpsimd.dma_start(out=xT_sb[:, 4:8, 0:512],
                                in_=xT_r[:, 4:8, 0:512])
            id16_sb = const.tile([16, 16], BF)
            nc.gpsimd.dma_start(out=id16_sb, in_=id16.ap())
            tri_sb = const.tile([128, 128], BF)
            nc.gpsimd.dma_start(out=tri_sb, in_=tri.ap())
            for s in range(1, 4):
                sl = slice(512 * s, 512 * s + 512)
                nc.sync.dma_start(out=xT_sb[:, 0:4, sl], in_=xT_r[:, 0:4, sl])
                nc.gpsimd.dma_start(out=xT_sb[:, 4:8, sl],
                                    in_=xT_r[:, 4:8, sl])
            w2s_sb = const.tile([128, D], BF)
            nc.sync.dma_start(out=w2s_sb, in_=w2s.ap())

            # persistent activation tensors
            # q128: q'_h at rows 32h+16..32h+32 (strip-aligned with kv128);
            #       rows 0:16 stage kv.
            # kv128: kv at rows 32h+16..32h+32, zeros elsewhere (scores lhsT).
            q128 = const.tile([128, T], BF)
            kv128 = const.tile([128, T], BF)
            kv_aug = const.tile([128, NB, 32], BF)  # col0 ones, 1:17 kvT, rest 1
            ZTs = const.tile([128, T], BF)
            outstage = const.tile([128, 16, D], BF)
            ones128 = const.tile([128, 512], BF)
            bpr128 = const.tile([128, 128], BF)
            wexp = const.tile([128, 16], BF)

            nc.vector.memset(kv128, 0.0)
            nc.vector.memset(ones128, 1.0)
            nc.vector.memset(bpr128, 0.0)
            nc.gpsimd.memset(kv_aug[:, :, 0:1], 1.0)
            nc.gpsimd.memset(kv_aug[:, :, 17:32], 1.0)
            # bias row for the proj ones-row matmul (row 0 = b' pattern)
            nc.sync.dma_start(out=bpr128[0:1, :], in_=bpr.ap())

            # ACT exp-table warm load + PE HAM warmup during input DMA
            nc.scalar.activation(wexp, wcq_sb[:, 0, 0:16], EXP)
            for w in range(8):
                wp = ps.tile([128, 512], F32, tag="po", bufs=2, name=f"w{w}")
                nc.tensor.matmul(wp, lhsT=wcq_sb[:, w % 8, :],
                                 rhs=wcq_sb[:, 0:4, :],
                                 start=True, stop=True)

            # ---- emission units ----
            def proj(s):
                """Project slab s (512 tokens): q' (+bias) for 4 heads, kv."""
                sl = slice(512 * s, 512 * s + 512)
                pp = ps.tile([128, 512], F32, tag="po", bufs=2, name=f"pp{s}")
                for kt in range(8):
                    nc.tensor.matmul(pp, lhsT=wcq_sb[:, kt, :],
                                     rhs=xT_sb[:, kt, sl],
                                     start=(kt == 0), stop=False)
                # bias via ones-row: lhsT row0 = b' pattern, rows 1:128 zero
                nc.tensor.matmul(pp, lhsT=bpr128, rhs=ones128,
                                 start=False, stop=True)
                nc.scalar.copy(out=q128[:, sl], in_=pp)
                # kv to the 4 head strips (sbuf->sbuf DMA)
                for h in range(4):
                    nc.sync.dma_start(
                        out=kv128[32 * h + 16:32 * h + 32, sl],
                        in_=q128[0:16, sl])

            def trans(s):
                """keys-major kv for the z matmuls, via PE transpose."""
                tp = ps.tile([128, 4, 16], BF, tag="po", bufs=2, name=f"tp{s}")
                for j in range(4):
                    b = 4 * s + j
                    nc.tensor.transpose(tp[:, j, :],
                                        in_=q128[0:16, 128 * b:128 * b + 128],
                                        identity=id16_sb)
                nc.scalar.copy(out=kv_aug[:, 4 * s:4 * s + 4, 1:17], in_=tp)

            scs = {}
            pxs = {}

            def sc_quad(qc, b):
                """Scores for all 4 heads of key block b, concurrently."""
                q0 = 512 * qc
                vb = max(0, 128 * b - q0)
                scA = ps.tile([128, 2, 512], F32, tag="sc", bufs=1,
                              name=f"scA{qc}_{b}")
                scB = ps.tile([128, 2, 512], F32, tag="sc2", bufs=1,
                              name=f"scB{qc}_{b}")
                scs[(qc, b)] = (scA, scB, vb)
                for h in range(4):
                    dst = (scA, scB)[h // 2]
                    nc.tensor.matmul(
                        dst[:, h % 2, vb:512],
                        lhsT=kv128[32 * h:32 * h + 32, 128 * b:128 * b + 128],
                        rhs=q128[32 * h:32 * h + 32, q0 + vb:q0 + 512],
                        start=True, stop=True,
                        tile_position=(32 * h, 0))

            def exp_unit(qc, b, half):
                """exp of one 2-head score unit; ACT for half 0, DVE half 1."""
                scA, scB, vb = scs[(qc, b)]
                src = (scA, scB)[half]
                px = pxp.tile([128, 2, 512], BF, tag="px",
                              name=f"px{qc}_{b}_{half}")
                pxs[(qc, b, half)] = px
                if half == 0:
                    nc.scalar.activation(px[:, :, vb:512], src[:, :, vb:512],
                                         EXP)
                else:
                    nc.vector.tensor_scalar(
                        out=px[:, :, vb:512].bitcast(U16),
                        in0=src[:, :, vb:512],
                        scalar1=SCH_A, scalar2=SCH_B,
                        op0=mybir.AluOpType.mult, op1=mybir.AluOpType.add)
                if b >= 4 * qc:  # diagonal block: causal stair mask (Pool)
                    trib = bass.AP(
                        tensor=tri_sb.tensor, offset=tri_sb.offset,
                        ap=[list(tri_sb.ap)[0], [0, 2], [1, 128]])
                    nc.gpsimd.tensor_mul(px[:, :, vb:vb + 128],
                                         px[:, :, vb:vb + 128], trib)

            def z_clear(qc):
                # zero-fill zps via a zero-weight matmul: sets has_written on
                # every element so the 4 concurrent col-tiled z matmuls can
                # all accumulate (start=False) without bank-clear races.
                zp = ps.tile([128, 512], F32, tag="zb", bufs=2,
                             name=f"zps{qc}")
                zps[qc] = zp
                nc.tensor.matmul(zp, lhsT=kv128[0:1, 0:128],
                                 rhs=ones128[0:1, :],
                                 start=True, stop=False)

            def z_quad(qc, b, blocks):
                q0 = 512 * qc
                vb = max(0, 128 * b - q0)
                pxA = pxs.pop((qc, b, 0))
                pxB = pxs.pop((qc, b, 1))
                zp = zps[qc]
                last = b == blocks - 1
                for h in range(4):
                    px = (pxA, pxB)[h // 2]
                    nc.tensor.matmul(
                        zp[32 * h:32 * h + 32, vb:512],
                        lhsT=kv_aug[:, b, :],
                        rhs=px[:, h % 2, vb:512],
                        start=False, stop=(last and h == 3),
                        tile_position=(0, 32 * h))

            def norm_recip(qc, c0=0, c1=512):
                zp = zps[qc]
                recf = work.tile([128, 512], F32, tag="recf",
                                 name=f"rf{qc}{c0}")
                nrms[(qc, c0)] = recf
                nc.vector.reciprocal_approx_fast(out=recf[:, 0:c1 - c0],
                                                 in_=zp[:, c0:c1])

            def norm_bcast(qc, c0=0, c1=512):
                recf = nrms[(qc, c0)]
                rbcs = work.tile([128, 512], F32, tag="rbcs",
                                 name=f"rs{qc}{c0}")
                # den recip sits at row 0 of each 32-row head strip
                nc.vector.stream_shuffle(rbcs[:, 0:c1 - c0],
                                         recf[:, 0:c1 - c0], mask=[0] * 32)
                nrms[(qc, c0)] = rbcs

            def norm_mul(qc, c0=0, c1=512):
                rbcs = nrms.pop((qc, c0))
                nc.vector.tensor_mul(ZTs[:, 512 * qc + c0:512 * qc + c1],
                                     zps[qc][:, c0:c1], rbcs[:, 0:c1 - c0])

            def outproj_mm(qc, m, n):
                qt = 4 * qc + m
                po = ps.tile([128, 512], F32, tag="po", bufs=2,
                             name=f"po{qc}{m}{n}")
                pos[(qt, n)] = po
                nc.tensor.matmul(
                    po, lhsT=ZTs[:, 128 * qt:128 * qt + 128],
                    rhs=w2s_sb[:, 512 * n:512 * n + 512],
                    start=True, stop=True)

            def outproj_drain(qc, m, n, eng):
                qt = 4 * qc + m
                po = pos.pop((qt, n))
                dst = outstage[:, qt, 512 * n:512 * n + 512]
                if eng == 0:
                    nc.scalar.copy(out=dst, in_=po)
                else:
                    nc.vector.tensor_copy(out=dst, in_=po)

            def outdma(qt0, qt1):
                out_r = outp.ap().rearrange("(m p) n -> p m n", p=128)
                nc.sync.dma_start(out=out_r[:, qt0:qt1, :],
                                  in_=outstage[:, qt0:qt1, :])

            # ---- schedule ----
            zps = {}
            nrms = {}
            pos = {}

            proj(0)
            trans(0)
            z_clear(0)

            for qc in range(4):
                blocks = 4 * qc + 4

                # filler units woven between score blocks, in order
                fillers = []
                if qc == 0:
                    fillers += [lambda: proj(1), lambda: trans(1),
                                lambda: proj(2), lambda: trans(2)]
                if qc == 1:
                    fillers += [lambda: proj(3), lambda: trans(3)]
                if qc >= 1:
                    q = qc - 1
                    fillers += [lambda q=q: norm_recip(q),
                                lambda q=q: norm_bcast(q),
                                lambda q=q: norm_mul(q)]
                    for m in range(4):
                        for n in range(2):
                            eng = 1 if (2 * m + n) % 5 == 4 else 0
                            fillers.append(
                                lambda q=q, m=m, n=n: outproj_mm(q, m, n))
                            fillers.append(
                                lambda q=q, m=m, n=n, e=eng:
                                outproj_drain(q, m, n, e))
                    fillers.append(lambda q=q: outdma(4 * q, 4 * q + 4))
                fillers.append(lambda q=qc: z_clear(q + 1) if q < 3 else None)

                stride = max(1, (2 * blocks) // max(1, len(fillers)))
                fi = 0
                ui = 0
                for b in range(blocks):
                    sc_quad(qc, b)
                    if b:
                        z_quad(qc, b - 1, blocks)
                    for half in range(2):
                        exp_unit(qc, b, half)
                        ui += 1
                        if fi < len(fillers) and ui % stride == 0:
                            fillers[fi]()
                            fi += 1
                z_quad(qc, blocks - 1, blocks)
                while fi < len(fillers):
                    fillers[fi]()
                    fi += 1

            # tail: final window norm + out-proj, split for pipelining
            norm_recip(3, 0, 256)
            norm_bcast(3, 0, 256)
            norm_mul(3, 0, 256)
            norm_recip(3, 256, 512)
            for m in (0, 1):
                for n in range(2):
                    outproj_mm(3, m, n)
                    outproj_drain(3, m, n, n)
            norm_bcast(3, 256, 512)
            norm_mul(3, 256, 512)
            outdma(12, 14)
            for m in (2, 3):
                for n in range(2):
                    outproj_mm(3, m, n)
                    outproj_drain(3, m, n, n)
            outdma(14, 16)

    nc.compile()
    return nc


def _prep_inputs(inputs):
    x = np.asarray(inputs["x"], np.float32)
    Wc = np.asarray(inputs["Wc"], np.float32)
    Wk = np.asarray(inputs["Wk"], np.float32)
    Wv = np.asarray(inputs["Wv"], np.float32)
    Wq = np.asarray(inputs["Wq"], np.float32)
    bq = np.asarray(inputs["bq"], np.float32)
    Wo = np.asarray(inputs["Wo"], np.float32)

    tri_np = np.triu(np.ones((128, 128), np.float32)).astype(BF16)
    id16_np = np.eye(16, dtype=BF16)

    xT_np = [np.ascontiguousarray(x[b].T).astype(BF16) for b in range(B)]

    in_maps = []
    for core in range(8):
        b, g = core // 4, core % 4
        # wcq cols: 32h+16..32h+32 = W'_h; cols 0:16 = Wc; rest zero
        wcq_np = np.zeros((D, 128), np.float32)
        wcq_np[:, 0:16] = Wc
        bprime_np = np.zeros((128,), np.float32)
        w2s_np = np.zeros((128, D), np.float32)
        for h in range(HPC):
            gh = 4 * g + h
            hs = slice(HD * gh, HD * gh + HD)
            wcq_np[:, 32 * h + 16:32 * h + 32] = (
                Wq[:, hs] * SCALE) @ Wk[:, hs].T
            bprime_np[32 * h + 16:32 * h + 32] = (bq[hs] * SCALE) @ Wk[:, hs].T
            w2s_np[32 * h + 1:32 * h + 17, :] = Wv[:, hs] @ Wo[hs, :]
        wcq_np = np.ascontiguousarray(
            wcq_np.reshape(8, 128, 128).transpose(1, 0, 2)).astype(BF16)
        in_maps.append({
            "xT": xT_np[b],
            "wcq": wcq_np,
            "bpr": bprime_np.reshape(1, 128).astype(BF16),
            "w2s": w2s_np.astype(BF16),
            "tri": tri_np,
            "id16": id16_np,
        })
    return in_maps


def run(inputs, trace=False, tmpdir=None):
    if "nc" not in _CACHE:
        _CACHE["nc"] = _build_program()
    nc = _CACHE["nc"]
    in_maps = _prep_inputs(inputs)

    kwargs = {}
    if trace:
        try:
            import antenv.axon_hooks  # noqa: F401
        except ImportError:
            import types
            import antenv  # noqa: F401
            from trn_agent_boot.trn_boot import _ntff_profile_via_ctypes
            hook = _ntff_profile_via_ctypes("/opt/axon/libaxon_pjrt.so")
            mod = types.ModuleType("antenv.axon_hooks")
            mod.get_axon_ntff_profile_hook = lambda: hook
            sys.modules["antenv.axon_hooks"] = mod
        kwargs = dict(trace=True, tmpdir=tmpdir)

    res = run_bass_kernel_spmd(nc, in_maps, list(range(8)), **kwargs)

    bc = np.asarray(inputs["bc"], np.float32)
    Wv = np.asarray(inputs["Wv"], np.float32)
    bv = np.asarray(inputs["bv"], np.float32)
    Wo = np.asarray(inputs["Wo"], np.float32)
    bo = np.asarray(inputs["bo"], np.float32)
    host_bias = bo + (bc @ Wv + bv) @ Wo

    out = np.zeros((B, T, D), np.float32)
    for core in range(8):
        out[core // 4] += res.results[core]["outp"].astype(np.float32)
    out += host_bias
    return out, res


def kernel(**inputs):
    out, _ = run(inputs, trace=False)
    return out
